# revision 1
# baseline (speedup 1.0000x reference)
"""Trainium2 Bass kernel for the CWICDense (conditional stripe matmul) module.

Problem (hardcoded shapes):
  x          [2, 512, 4096] f32    tokens T=1024, features I=4096
  W_kernel   [4096, 4096]   f32    viewed as [I, N=32 stripes, Q=128]
  thresholds [4096, 32]     f32
  mu         [4096]         f32    (structurally zero in this module)
  out_mu     [4096]         f32
  where      [2, 512]       bool   (unused by the reference computation)

  y[t, n*Q+q] = sum_i x_off[t,i] * (|x_off[t,i]| >= thresholds[i,n]) * W[i, n*Q+q]
                + out_mu[n*Q+q]

Sharding across 8 NeuronCores: 8-way tensor parallel over stripes (4 stripes
= 512 out cols per core); every core sees all 1024 tokens.

Gate strategy (vs the 170us custom-op baseline): stock DVE ops on fp16 —
per (k-tile, stripe pair): two tensor_scalar is_ge masks (a16 >= t', 4x
DVE mode, ~470ns each; the scalar MUST be a [P,1]-contiguous tile) plus
one tensor_tensor mult z = m * [x|x] (2x mode, ~1.2us) against an x tile
DMA'd twice from HBM so the multiply runs on a flat 2D pattern. fp16
matmuls (full PE rate) accumulate y^T in fp32 PSUM.

Exactness: the host nudges |x16| by -1 ulp on ~1e3 entries and picks
per-(i,n) fp32 thresholds in the exact-gate interval (lo, hi] so fp16
comparisons reproduce the fp32 gate (|x32| >= thr) bit-exactly. Remaining
error: fp16 rounding of the matmul inputs, ~5e-4.
"""

import sys

if "/opt/trn_rl_repo" not in sys.path:
    sys.path.insert(0, "/opt/trn_rl_repo")

import numpy as np

import concourse.bass as bass
import concourse.mybir as mybir
import concourse.tile as tile
from concourse import bacc, bass_utils
from concourse import dve_ops as _dve_ops
from concourse.dve_spec import Spec, Src0, C0, C1, Zero, select, lower, PageIdx
from concourse.dve_spec import _has_src1
from concourse.dve_uop import DveOpSpec

# ---- problem constants -------------------------------------------------
B, S, I, N, Q = 2, 512, 4096, 32, 128
T = B * S                 # 1024 tokens
OUT = N * Q               # 4096
NCORES = 8
NS = N // NCORES          # 4 stripes per core
OUT_C = NS * Q            # 512 out cols per core
KT = I // 128             # 32 contraction tiles
P = 128
HF = 2                    # token halves per matmul group (1024 -> 2 x 512)
TH = T // HF              # 512
PAIRS = NS // 2

STOCK_K0 = 0              # all k-tiles use the stock mask+apply path

_CACHE = {}


def _gate2_ref(in0, in1, s0, s1, imm2):
    steps = np.arange(in0.shape[1], dtype=np.float32)[None, :, None]
    t = (s0[:, :, None].astype(np.float32)
         + steps * s1[:, :, None].astype(np.float32)).astype(np.float32)
    return np.where((in0 >= t) | (in0 <= -t), in0, 0.0).astype(in0.dtype)


def _register_op(name, spec, subdim):
    if name in _dve_ops._SUB_OPCODE_FOR_NAME:
        return next(op for op in _dve_ops.OPS if op.name == name)
    row = max(_dve_ops._SUB_OPCODE_FOR_NAME.values()) + 1
    assert row < 0x20
    _dve_ops._SUB_OPCODE_FOR_NAME[name] = row
    shas = {}
    for ver in ("v3",):
        tmp = DveOpSpec(
            name=name, opcode=row, uops=lower(spec, ver=ver),
            rd1_en=_has_src1(spec),
        )
        shas[ver] = tmp.sha(ver)
    op = _dve_ops.DveOp(name, spec, subdim=subdim, uops_sha=shas)
    _dve_ops.OPS.append(op)
    _dve_ops.CUSTOM_DVE_SPECS[name] = spec
    return op


def _register_gate_op():
    pg = PageIdx(C0, C1)
    spec = Spec(
        body=select((Src0 >= pg) | (Src0 <= Zero - pg), Src0, Zero),
        reference=_gate2_ref,
    )
    return _register_op("CWIC_GATE2_ANT", spec, subdim=True)


def _build():
    f32 = mybir.dt.float32
    f16 = mybir.dt.float16
    A = mybir.AluOpType
    gate_op = _register_gate_op()
    nc = bacc.Bacc("TRN2", target_bir_lowering=False, debug=False)

    xT_d = nc.dram_tensor("xT", [I, T], f16, kind="ExternalInput").ap()
    aT_d = nc.dram_tensor("aT", [I, T], f16, kind="ExternalInput").ap()
    w_d = nc.dram_tensor("w", [I, OUT_C], f16, kind="ExternalInput").ap()
    # thr: [t0, dt01, t2, dt23] pair encoding; thrp: plain per-stripe t'
    thr_d = nc.dram_tensor("thr", [I, NS], f32, kind="ExternalInput").ap()
    thrp_d = nc.dram_tensor("thrp", [I, NS], f32, kind="ExternalInput").ap()
    thrs_d = nc.dram_tensor("thrs", [I, NS], f32, kind="ExternalInput").ap()
    mu_d = nc.dram_tensor("mu", [P, NS], f32, kind="ExternalInput").ap()
    yT_d = nc.dram_tensor("yT", [OUT_C, T], f32, kind="ExternalOutput").ap()

    w_v = w_d.rearrange("(k p) c -> p k c", p=P)

    with tile.TileContext(nc) as tc:
        with (
            tc.tile_pool(name="const", bufs=1) as constp,
            tc.tile_pool(name="xT", bufs=18) as xTp,
            tc.tile_pool(name="aT", bufs=8) as aTp,
            tc.tile_pool(name="thr", bufs=8 * KT) as thrp,
            tc.tile_pool(name="w", bufs=12) as wp,
            tc.tile_pool(name="m", bufs=8) as mp_,
            tc.tile_pool(name="z", bufs=8) as zp,
            tc.tile_pool(name="yT", bufs=8) as yTp,
            tc.tile_pool(name="acc", bufs=NS * HF, space="PSUM") as accp,
        ):
            # Each per-partition scalar gets its own [P,1]-contiguous tile —
            # a column slice of a wider tile drops the DVE to 1/4 rate.
            xT = []
            aT = {}
            thrT = []
            thrP = []
            thrS = []

            def load_k(k):
                pl = {}
                sg = {}
                for j in range(NS):
                    if j % 2 == 0:
                        # this stripe's mask runs on ACT (sigmoid bias tile)
                        bj = thrp.tile([P, 1], f32, tag="thr",
                                       name=f"ts{k}_{j}")
                        nc.sync.dma_start(
                            bj[:], thrs_d[k * P:(k + 1) * P, j:j + 1])
                        sg[j] = bj
                        continue
                    pj = thrp.tile([P, 1], f32, tag="thr",
                                   name=f"tp{k}_{j}")
                    nc.gpsimd.dma_start(
                        pj[:], thrp_d[k * P:(k + 1) * P, j:j + 1])
                    pl[j] = pj
                thrS.append(sg)
                ak = aTp.tile([P, T], f16, tag="aT", name=f"ak{k}")
                nc.sync.dma_start(ak[:], aT_d[k * P:(k + 1) * P, :])
                aT[k] = ak
                thrP.append(pl)
                xk = xTp.tile([P, T], f16, tag="xT", name=f"xk{k}")
                nc.sync.dma_start(xk[:], xT_d[k * P:(k + 1) * P, :])
                xT.append(xk)

            KC = 8
            wcs = {}

            def load_w(r):
                # W chunk triggers ride the gpsimd DGE queue, keeping the
                # ACT queue free for mask activations
                for n in range(NS):
                    wc = wp.tile([P, KC * Q], f16, tag="w", name=f"wc{n}_{r}")
                    nc.gpsimd.dma_start(
                        wc[:].rearrange("p (k q) -> p k q", q=Q),
                        w_v[:, r * KC:(r + 1) * KC, n * Q:(n + 1) * Q],
                    )
                    wcs[(n, r)] = wc

            load_w(0)
            for k in range(2):
                load_k(k)

            # HAM warm-up
            warmsrc = constp.tile([P, TH], f16, tag="warmsrc")
            nc.scalar.dma_start(warmsrc[:], xT_d[P:2 * P, 0:TH])

            for k in range(2, KT):
                load_k(k)
            for r in range(1, KT // KC):
                load_w(r)

            mu_sb = constp.tile([P, NS], f32, tag="mu")
            nc.scalar.dma_start(mu_sb[:], mu_d)

            accs = [
                accp.tile([P, TH], f32, tag="acc", name=f"acc{n}_{h}")
                for n in range(NS) for h in range(HF)
            ]
            for _ in range(6):
                nc.tensor.matmul(
                    accs[-1][:],
                    warmsrc[:, 0:P],
                    warmsrc[:],
                    start=True,
                    stop=True,
                )

            for k in range(KT):
                xk = xT[k][:]
                x_pg = bass.AP(xk.tensor, xk.offset,
                               [list(xk.ap[0]), [0, 2], list(xk.ap[1])])
                for pair in range(PAIRS):
                    zt = zp.tile([P, 2 * T], f16, tag="z")
                    # 2 flat is_ge masks + 1 flat mult over [x|x]
                    m2 = mp_.tile([P, 2 * T], f16, tag="m")
                    for s in range(2):
                        n = 2 * pair + s
                        if s == 0:
                            # mask on the otherwise-idle ACT engine:
                            # sigmoid(1e30*(a - mid)) saturates to 1.0/0.0
                            # exactly (a is never inside the gate interval)
                            nc.scalar.activation(
                                m2[:, s * T:(s + 1) * T], aT[k][:],
                                mybir.ActivationFunctionType.Sigmoid,
                                bias=thrS[k][n][:],
                                scale=1e30,
                            )
                        else:
                            nc.vector.tensor_scalar(
                                m2[:, s * T:(s + 1) * T], aT[k][:],
                                thrP[k][n][:], None,
                                op0=A.is_ge,
                            )
                    nc.vector.tensor_tensor(
                        zt[:].rearrange("p (s t) -> p s t", s=2),
                        m2[:].rearrange("p (s t) -> p s t", s=2),
                        x_pg,
                        op=A.mult,
                    )
                    for s in range(2):
                        n = 2 * pair + s
                        for h in range(HF):
                            nc.tensor.matmul(
                                accs[n * HF + h][:],
                                wcs[(n, k // KC)][:, (k % KC) * Q:
                                                  (k % KC + 1) * Q],
                                zt[:, s * T + h * TH:s * T + (h + 1) * TH],
                                start=(k == 0),
                                stop=(k == KT - 1),
                            )
            for n in range(NS):
                for h in range(HF):
                    yt = yTp.tile([P, TH], f32, tag="yT")
                    if (n * HF + h) % 2 == 0:
                        nc.scalar.activation(
                            yt[:], accs[n * HF + h][:],
                            mybir.ActivationFunctionType.Identity,
                            bias=mu_sb[:, n:n + 1],
                        )
                    else:
                        nc.vector.tensor_scalar(
                            yt[:], accs[n * HF + h][:], mu_sb[:, n:n + 1],
                            None, op0=A.add,
                        )
                    nc.sync.dma_start(
                        yT_d[n * P:(n + 1) * P, h * TH:(h + 1) * TH], yt[:]
                    )
    nc.compile()
    return nc


def _get_nc():
    if "nc" not in _CACHE:
        _CACHE["nc"] = _build()
    return _CACHE["nc"]


def _interval_dt(s0, lo, hi):
    """fp32 dt with fl(s0 + dt) in (lo, hi] (monotone ulp adjustment)."""
    tgt = np.where(np.isfinite(hi), hi,
                   np.nextafter(lo, np.float32(np.inf))).astype(np.float32)
    dt = (tgt - s0).astype(np.float32)
    for _ in range(64):
        s = (s0 + dt).astype(np.float32)
        bad_hi = s > hi
        bad_lo = s <= lo
        if not (bad_hi | bad_lo).any():
            return dt
        dt = np.where(bad_hi, np.nextafter(dt, np.float32(-np.inf)),
                      dt).astype(np.float32)
        dt = np.where(bad_lo, np.nextafter(dt, np.float32(np.inf)),
                      dt).astype(np.float32)
    raise AssertionError("stripe-pair threshold delta not reachable")


def _prep_gate(xT32, thr):
    """fp16 |x| with -1ulp nudges and per-(i,n) exact-gate intervals."""
    a32 = np.abs(xT32)                      # [I, T]
    a16 = a32.astype(np.float16)
    INF16 = np.float16(np.inf)

    CH = 512
    hi = np.empty((I, N), np.float16)
    lo = np.empty((I, N), np.float16)

    def pass_hilo(rows):
        p32 = a32[rows, None, :] >= thr[rows, :, None]
        a16b = np.broadcast_to(a16[rows, None, :], p32.shape)
        hi[rows] = np.where(p32, a16b, INF16).min(axis=2)
        lo[rows] = np.where(~p32, a16b, -INF16).max(axis=2)
        return p32

    rows_all = np.arange(I)
    for c in range(0, I, CH):
        pass_hilo(rows_all[c:c + CH])

    for _ in range(12):
        coll = hi <= lo
        bad_rows = np.nonzero(coll.any(axis=1))[0]
        if bad_rows.size == 0:
            break
        p32 = a32[bad_rows, None, :] >= thr[bad_rows, :, None]
        nudge = ((~p32) & coll[bad_rows, :, None]
                 & (a16[bad_rows, None, :] == hi[bad_rows, :, None]))
        nudge_it = nudge.any(axis=1)
        a16[bad_rows] = np.where(
            nudge_it, np.nextafter(a16[bad_rows], np.float16(-np.inf)),
            a16[bad_rows])
        pass_hilo(bad_rows)
    else:
        raise AssertionError("fp16 gate nudging did not converge")

    hi32 = hi.astype(np.float32)
    lo32 = lo.astype(np.float32)
    x16 = np.where(xT32 >= 0, a16, -a16).astype(np.float16)
    return x16, a16, lo32, hi32


def _make_in_maps(x, W_kernel, thresholds, mu, out_mu):
    xf = np.asarray(x, dtype=np.float32).reshape(T, I)
    xf = xf - np.asarray(mu, dtype=np.float32)[None, :]
    xT = np.ascontiguousarray(xf.T)
    thr = np.asarray(thresholds, np.float32)
    x16, a16, lo32, hi32 = _prep_gate(xT, thr)
    W16 = np.asarray(W_kernel, np.float32).astype(np.float16)
    omu = np.asarray(out_mu, np.float32)
    tplain = np.where(np.isfinite(hi32), hi32,
                      np.nextafter(lo32, np.float32(np.inf))
                      ).astype(np.float32)
    with np.errstate(invalid="ignore"):
        tsig = (np.float32(-1e30)
                * ((lo32 + hi32) * np.float32(0.5))).astype(np.float32)
    # all-pass rows have lo=-inf -> mid=-inf -> bias=+inf (sigmoid -> 1) ok;
    # no-pass rows (hi=+inf) analogously give -inf (sigmoid -> 0)
    tsig = np.where(np.isnan(tsig), np.float32(0.0), tsig)
    in_maps = []
    for g in range(NCORES):
        lo_c = lo32[:, g * NS:(g + 1) * NS]
        hi_c = hi32[:, g * NS:(g + 1) * NS]
        tp_c = tplain[:, g * NS:(g + 1) * NS]
        cols = []
        for pair in range(PAIRS):
            s0 = tp_c[:, 2 * pair]
            cols += [s0, _interval_dt(s0, lo_c[:, 2 * pair + 1],
                                      hi_c[:, 2 * pair + 1])]
        in_maps.append({
            "xT": x16,
            "aT": a16,
            "w": np.ascontiguousarray(W16[:, g * OUT_C:(g + 1) * OUT_C]),
            "thr": np.ascontiguousarray(np.stack(cols, axis=1)),
            "thrp": np.ascontiguousarray(tp_c),
            "thrs": np.ascontiguousarray(tsig[:, g * NS:(g + 1) * NS]),
            "mu": np.ascontiguousarray(
                omu[g * OUT_C:(g + 1) * OUT_C].reshape(NS, P).T
            ),
        })
    return in_maps


def _assemble(results):
    yT = np.concatenate([results[g]["yT"] for g in range(NCORES)], axis=0)
    return np.ascontiguousarray(yT.T).reshape(B, S, OUT)


def run(inputs, **spmd_kwargs):
    nc = _get_nc()
    in_maps = _make_in_maps(
        inputs["x"], inputs["W_kernel"], inputs["thresholds"],
        inputs["mu"], inputs["out_mu"],
    )
    res = bass_utils.run_bass_kernel_spmd(
        nc, in_maps, core_ids=list(range(NCORES)), **spmd_kwargs
    )
    return _assemble(res.results), res


def kernel(x, W_kernel, thresholds, mu, out_mu, where):
    y, _ = run({
        "x": x, "W_kernel": W_kernel, "thresholds": thresholds,
        "mu": mu, "out_mu": out_mu, "where": where,
    })
    return y



# revision 2
# speedup vs baseline: 1.3573x; 1.3573x over previous
"""Trainium2 Bass kernel for the CWICDense (conditional stripe matmul) module.

Problem (hardcoded shapes):
  x          [2, 512, 4096] f32    tokens T=1024, features I=4096
  W_kernel   [4096, 4096]   f32    viewed as [I, N=32 stripes, Q=128]
  thresholds [4096, 32]     f32
  mu         [4096]         f32    (structurally zero in this module)
  out_mu     [4096]         f32
  where      [2, 512]       bool   (unused by the reference computation)

  y[t, n*Q+q] = sum_i x_off[t,i] * (|x_off[t,i]| >= thresholds[i,n]) * W[i, n*Q+q]
                + out_mu[n*Q+q]

Sharding across 8 NeuronCores: 8-way tensor parallel over stripes (4 stripes
= 512 out cols per core); every core sees all 1024 tokens.

Gate strategy: one custom DVE op per (k-tile, stripe) computing
  z = select((x >= t') | (x <= -t'), x, 0)
directly from signed fp16 x with per-partition fp32 thresholds, using a
hand-written 2X_1PORT uop program (~724ns per [128,1024] tile vs ~1050ns
for the old is_ge+mult split).  No |x| tensor, no ACT sigmoid masks.

Exactness: the host nudges |x16| by -1 ulp on ~1e3 entries and picks
per-(i,n) fp32 thresholds t' in the exact-gate interval (lo, hi] so the
fp16 comparisons reproduce the fp32 gate (|x32| >= thr) bit-exactly.
Remaining error: fp16 rounding of the matmul inputs, ~3e-4.
"""

import sys

if "/opt/trn_rl_repo" not in sys.path:
    sys.path.insert(0, "/opt/trn_rl_repo")
if "/root/problem" not in sys.path:
    sys.path.insert(0, "/root/problem")

import numpy as np

import concourse.bass as bass
import concourse.mybir as mybir
import concourse.tile as tile
from concourse import bacc, bass_utils

# ---- custom gate op (inlined so kernel.py is self-contained) -----------
from dataclasses import dataclass

from concourse import dve_ops as _dve_ops
from concourse.dve_spec import Spec, Src0, Src1, C0, C1, select, lower
from concourse.dve_spec import _has_src1
from concourse.dve_uop import (
    AluInp,
    AluOp,
    DelayInp,
    DveOpSpec,
    InpSel,
    OutPath,
    OutSel,
    Trigger,
    UopConfig,
)

_FULL_SPECS = {}


@dataclass(frozen=True)
class _DveOpHand(_dve_ops.DveOp):
    def compile(self, ver):
        return _FULL_SPECS[(self.name, ver)]


def _gate1_ref(in0, in1, s0, s1, imm2):
    x = in0.astype(np.float32)
    return np.where((x >= s0) | (x <= s1), x,
                    in1.astype(np.float32) - in1.astype(np.float32)
                    ).astype(np.float32)


def _build_2x_uop():
    u = UopConfig()
    u.enable_input(InpSel.SRC_0, 1)      # d0 = x_lo
    u.enable_input(InpSel.CONST_0, 2)    # d1 = t'
    u.enable_input(InpSel.CONST_1, 3)    # d2 = -t'
    u.enable_input(InpSel.SRC_0_HI, 4)   # d3 = x_hi
    u.require_inp0 = 1
    u.require_inp1 = 1
    u.trigger = (Trigger.SRC_TENSOR_DONE, Trigger.NONE, Trigger.NONE)
    u.next_uop = (0, 0, 0)

    b = u.datapath_config
    b[0].enable_alu(AluOp.IS_GE, AluInp.PREV_DELAY_0, AluInp.PREV_DELAY_1)
    b[0].pass_through_delay(0, 1, 2, 3)
    b[1].enable_alu(AluOp.IS_LE, AluInp.PREV_DELAY_0, AluInp.PREV_DELAY_2)
    b[1].pass_through_delay(0, 1, 2, 3)
    b[1].enable_delay_from_src(DelayInp.PREV_ALU_OUT, 4)
    b[2].enable_alu(AluOp.LOGICAL_OR, AluInp.PREV_ALU_OUT, AluInp.PREV_DELAY_4)
    b[2].pass_through_delay(0, 1, 2, 3)
    b[3].enable_alu(AluOp.MULTIPLY, AluInp.PREV_ALU_OUT, AluInp.PREV_DELAY_0)
    b[3].pass_through_delay(1, 2, 3)
    b[4].enable_alu(AluOp.IS_GE, AluInp.PREV_DELAY_3, AluInp.PREV_DELAY_1)
    b[4].enable_delay_from_src(DelayInp.PREV_ALU_OUT, 0)
    b[4].pass_through_delay(2, 3)
    b[5].enable_alu(AluOp.IS_LE, AluInp.PREV_DELAY_3, AluInp.PREV_DELAY_2)
    b[5].enable_delay_from_src(DelayInp.PREV_ALU_OUT, 1)
    b[5].pass_through_delay(0, 3)
    b[6].enable_alu(AluOp.LOGICAL_OR, AluInp.PREV_ALU_OUT, AluInp.PREV_DELAY_1)
    b[6].pass_through_delay(0, 3)
    b[7].enable_alu(AluOp.MULTIPLY, AluInp.PREV_ALU_OUT, AluInp.PREV_DELAY_3)
    b[7].pass_through_delay(0)

    u.enable_output(OutSel.DELAY_0, OutPath.WR0_LO)   # z_lo
    u.enable_output(OutSel.ALU_OUT, OutPath.WR0_HI)   # z_hi
    return u


def _register_gate1():
    name = "CWIC_GATE1_2X_ANT"
    spec = Spec(
        body=select((Src0 >= C0) | (Src0 <= C1), Src0, Src1 - Src1),
        reference=_gate1_ref,
    )
    if name in _dve_ops._SUB_OPCODE_FOR_NAME:
        return next(op for op in _dve_ops.OPS if op.name == name)
    row = max(_dve_ops._SUB_OPCODE_FOR_NAME.values()) + 1
    assert row < 0x20
    _dve_ops._SUB_OPCODE_FOR_NAME[name] = row
    shas = {}
    for ver in ("v3",):
        full = DveOpSpec(
            name=name,
            opcode=row,
            uops=lower(spec, ver=ver),
            uops_2x=[_build_2x_uop()],
            rd1_en=_has_src1(spec),
            perf_max=1,
        )
        full.validate(ver)
        _FULL_SPECS[(name, ver)] = full
        shas[ver] = full.sha(ver)
    op = _DveOpHand(name, spec, subdim=False, uops_sha=shas)
    _dve_ops.OPS.append(op)
    _dve_ops.CUSTOM_DVE_SPECS[name] = spec
    return op


def _emit_gate1(nc, op, out, x, t_pos, t_neg):
    """z = x * ((x >= t_pos) | (x <= t_neg)); 2X_1PORT fp16."""
    bi = nc.vector._custom_dve(op, out=out, in0=x, in1=x, s0=t_pos, s1=t_neg)
    bi.ins.perf_max = 1
    return bi


# ---- problem constants -------------------------------------------------
B, S, I, N, Q = 2, 512, 4096, 32, 128
T = B * S                 # 1024 tokens
OUT = N * Q               # 4096
NCORES = 8
NS = N // NCORES          # 4 stripes per core
OUT_C = NS * Q            # 512 out cols per core
KT = I // 128             # 32 contraction tiles
P = 128
HF = 2                    # token halves per matmul group (1024 -> 2 x 512)
TH = T // HF              # 512
KC = 8                    # k-tiles per W chunk DMA
RT = KT // KC             # 4 chunk rounds

_CACHE = {}


def _build():
    f32 = mybir.dt.float32
    f16 = mybir.dt.float16
    gate_op = _register_gate1()
    nc = bacc.Bacc("TRN2", target_bir_lowering=False, debug=False)

    xT_d = nc.dram_tensor("xT", [I, T], f16, kind="ExternalInput").ap()
    # w host layout: [NS, RT, P, KC*Q] so each (n, r) chunk is contiguous
    w_d = nc.dram_tensor("w", [NS * RT * P, KC * Q], f16,
                         kind="ExternalInput").ap()
    # thr columns per k-row-block: (t'_0, -t'_0, t'_1, -t'_1, ...)
    thr_d = nc.dram_tensor("thr", [I, 2 * NS], f32, kind="ExternalInput").ap()
    mu_d = nc.dram_tensor("mu", [P, NS], f32, kind="ExternalInput").ap()
    yT_d = nc.dram_tensor("yT", [OUT_C, T], f32, kind="ExternalOutput").ap()

    w_v = w_d.rearrange("(n r p) c -> n r p c", n=NS, r=RT)

    with tile.TileContext(nc) as tc:
        with (
            tc.tile_pool(name="const", bufs=1) as constp,
            tc.tile_pool(name="xT", bufs=KT) as xTp,
            tc.tile_pool(name="thr", bufs=KT) as thrp,
            tc.tile_pool(name="w", bufs=NS * RT) as wp,
            tc.tile_pool(name="z", bufs=8) as zp,
            tc.tile_pool(name="yT", bufs=8) as yTp,
            tc.tile_pool(name="acc", bufs=NS * HF, space="PSUM") as accp,
        ):
            xT = []
            thrT = []

            def load_k(k):
                tk = thrp.tile([P, 2 * NS], f32, tag="thr", name=f"thr{k}")
                nc.gpsimd.dma_start(tk[:], thr_d[k * P:(k + 1) * P, :])
                thrT.append(tk)
                xk = xTp.tile([P, T], f16, tag="xT", name=f"xk{k}")
                nc.sync.dma_start(xk[:], xT_d[k * P:(k + 1) * P, :])
                xT.append(xk)

            wcs = {}

            def load_w(r):
                for n in range(NS):
                    wc = wp.tile([P, KC * Q], f16, tag="w", name=f"wc{n}_{r}")
                    nc.gpsimd.dma_start(wc[:], w_v[n, r])
                    wcs[(n, r)] = wc

            load_w(0)
            for k in range(2):
                load_k(k)

            # ACT warm-up: trigger the activation-table load early so the
            # final bias-adds don't pay it.
            mu_sb = constp.tile([P, NS], f32, tag="mu")
            nc.scalar.dma_start(mu_sb[:], mu_d)
            warm = constp.tile([P, 1], f32, tag="warm")
            nc.scalar.activation(
                warm[:], mu_sb[:, 0:1],
                mybir.ActivationFunctionType.Identity,
            )

            for k in range(2, KT):
                load_k(k)
            for r in range(1, RT):
                load_w(r)

            # PE warm-up (HAM)
            warmsrc = constp.tile([P, TH], f16, tag="warmsrc")
            nc.scalar.dma_start(warmsrc[:], xT_d[P:2 * P, 0:TH])

            accs = [
                accp.tile([P, TH], f32, tag="acc", name=f"acc{n}_{h}")
                for n in range(NS) for h in range(HF)
            ]
            for _ in range(6):
                nc.tensor.matmul(
                    accs[-1][:],
                    warmsrc[:, 0:P],
                    warmsrc[:],
                    start=True,
                    stop=True,
                )

            for k in range(KT):
                xk = xT[k][:]
                for n in range(NS):
                    zt = zp.tile([P, T], f16, tag="z")
                    _emit_gate1(
                        nc, gate_op, zt[:], xk,
                        thrT[k][:, 2 * n:2 * n + 1],
                        thrT[k][:, 2 * n + 1:2 * n + 2],
                    )
                    for h in range(HF):
                        nc.tensor.matmul(
                            accs[n * HF + h][:],
                            wcs[(n, k // KC)][:, (k % KC) * Q:
                                              (k % KC + 1) * Q],
                            zt[:, h * TH:(h + 1) * TH],
                            start=(k == 0),
                            stop=(k == KT - 1),
                        )

            for n in range(NS):
                for h in range(HF):
                    yt = yTp.tile([P, TH], f32, tag="yT")
                    nc.scalar.activation(
                        yt[:], accs[n * HF + h][:],
                        mybir.ActivationFunctionType.Identity,
                        bias=mu_sb[:, n:n + 1],
                    )
                    nc.sync.dma_start(
                        yT_d[n * P:(n + 1) * P, h * TH:(h + 1) * TH], yt[:]
                    )
    nc.compile()
    return nc


def _get_nc():
    if "nc" not in _CACHE:
        _CACHE["nc"] = _build()
    return _CACHE["nc"]


def _prep_gate(xT32, thr):
    """fp16 x with -1 ulp nudges on |x| and per-(i,n) exact-gate intervals.

    Returns x16 and fp32 thresholds tplain with: |x16| >= tplain  <=>
    |x32| >= thr, elementwise-exactly."""
    a32 = np.abs(xT32)                      # [I, T]
    a16 = a32.astype(np.float16)
    INF16 = np.float16(np.inf)

    CH = 512
    hi = np.empty((I, N), np.float16)
    lo = np.empty((I, N), np.float16)

    def pass_hilo(rows):
        p32 = a32[rows, None, :] >= thr[rows, :, None]
        a16b = np.broadcast_to(a16[rows, None, :], p32.shape)
        hi[rows] = np.where(p32, a16b, INF16).min(axis=2)
        lo[rows] = np.where(~p32, a16b, -INF16).max(axis=2)
        return p32

    rows_all = np.arange(I)
    for c in range(0, I, CH):
        pass_hilo(rows_all[c:c + CH])

    for _ in range(12):
        coll = hi <= lo
        bad_rows = np.nonzero(coll.any(axis=1))[0]
        if bad_rows.size == 0:
            break
        p32 = a32[bad_rows, None, :] >= thr[bad_rows, :, None]
        nudge = ((~p32) & coll[bad_rows, :, None]
                 & (a16[bad_rows, None, :] == hi[bad_rows, :, None]))
        nudge_it = nudge.any(axis=1)
        a16[bad_rows] = np.where(
            nudge_it, np.nextafter(a16[bad_rows], np.float16(-np.inf)),
            a16[bad_rows])
        pass_hilo(bad_rows)
    else:
        raise AssertionError("fp16 gate nudging did not converge")

    hi32 = hi.astype(np.float32)
    lo32 = lo.astype(np.float32)
    x16 = np.where(xT32 >= 0, a16, -a16).astype(np.float16)
    tplain = np.where(np.isfinite(hi32), hi32,
                      np.nextafter(lo32, np.float32(np.inf))
                      ).astype(np.float32)
    return x16, tplain


def _make_in_maps(x, W_kernel, thresholds, mu, out_mu):
    xf = np.asarray(x, dtype=np.float32).reshape(T, I)
    xf = xf - np.asarray(mu, dtype=np.float32)[None, :]
    xT = np.ascontiguousarray(xf.T)
    thr = np.asarray(thresholds, np.float32)
    x16, tplain = _prep_gate(xT, thr)
    W16 = np.asarray(W_kernel, np.float32).astype(np.float16)
    omu = np.asarray(out_mu, np.float32)
    in_maps = []
    for g in range(NCORES):
        tp_c = tplain[:, g * NS:(g + 1) * NS]            # [I, NS]
        thr2 = np.empty((I, 2 * NS), np.float32)
        thr2[:, 0::2] = tp_c
        thr2[:, 1::2] = -tp_c
        # w chunks: [NS, RT, P, KC*Q]; element (n,r,p,kq) =
        #   W16[(r*KC + k)*P + p, g*OUT_C + n*Q + q]
        wg = W16[:, g * OUT_C:(g + 1) * OUT_C]           # [I, OUT_C]
        wr = wg.reshape(RT, KC, P, NS, Q)                # [r, k, p, n, q]
        wa = np.ascontiguousarray(
            wr.transpose(3, 0, 2, 1, 4).reshape(NS * RT * P, KC * Q)
        )
        in_maps.append({
            "xT": x16,
            "w": wa,
            "thr": np.ascontiguousarray(thr2),
            "mu": np.ascontiguousarray(
                omu[g * OUT_C:(g + 1) * OUT_C].reshape(NS, P).T
            ),
        })
    return in_maps


def _assemble(results):
    yT = np.concatenate([results[g]["yT"] for g in range(NCORES)], axis=0)
    return np.ascontiguousarray(yT.T).reshape(B, S, OUT)


def run(inputs, **spmd_kwargs):
    nc = _get_nc()
    in_maps = _make_in_maps(
        inputs["x"], inputs["W_kernel"], inputs["thresholds"],
        inputs["mu"], inputs["out_mu"],
    )
    res = bass_utils.run_bass_kernel_spmd(
        nc, in_maps, core_ids=list(range(NCORES)), **spmd_kwargs
    )
    return _assemble(res.results), res


def kernel(x, W_kernel, thresholds, mu, out_mu, where):
    y, _ = run({
        "x": x, "W_kernel": W_kernel, "thresholds": thresholds,
        "mu": mu, "out_mu": out_mu, "where": where,
    })
    return y
